# revision 35
# baseline (speedup 1.0000x reference)
"""Trainium2 Bass kernel for nn_GraphTransformerPE.

Sharding: graph-data-parallel. 16 graphs x 420 nodes; core c owns graphs
(2c, 2c+1). Weights replicated, no cross-core traffic; host slices inputs,
precomputes hT = (x + node/lobe/lung PE)^T and the per-graph edge-count
matrices M, pre-swizzles all weights into their SBUF slab layouts (all in
bf16), and concatenates the per-core [2,18] outputs.

Device formulation: per-graph DENSE attention. M is the 420x420 edge
multiplicity matrix, then TransformerConv softmax-aggregation ==
  w = M * exp(S/sqrt(d) - rowmax),  A = w / (rowsum(w)+1e-16),
  msg = A @ V  (computed transposed),
which reproduces segment softmax exactly. Matmuls use bf16 stationary
operands (weights / hT / vt) to enable fast-weight-load; accumulation is
always fp32 in PSUM. Biases are all zero in this model and skipped.

Layout: activations feature-major (transposed): hT [2048,840] bf16 feeds
every projection; conv outputs produced directly transposed (r1T
[2048,840] bf16, r2T [64,840]); fc1 weights are prefetched into SBUF
(bf16) during conv1/conv2 so the fc head runs without DMA waits.
"""

import sys
import types
from contextlib import ExitStack

import numpy as np
import ml_dtypes

# ---- NTFF profile hook shim (antenv.axon_hooks absent in this image) ----
if "antenv.axon_hooks" not in sys.modules:
    _m = types.ModuleType("antenv.axon_hooks")
    _hook = [None]
    _m.set_axon_ntff_profile_hook = lambda h: _hook.__setitem__(0, h)
    _m.get_axon_ntff_profile_hook = lambda: _hook[0]
    sys.modules["antenv.axon_hooks"] = _m
    try:
        from trn_agent_boot.trn_boot import _ntff_profile_via_ctypes
        _m.set_axon_ntff_profile_hook(
            _ntff_profile_via_ctypes("/opt/axon/libaxon_pjrt.so"))
    except Exception:
        pass

import concourse.bacc as bacc
import concourse.tile as tile
from concourse import bass_utils, mybir

F32 = mybir.dt.float32
F32R = mybir.dt.float32r
BF16 = mybir.dt.bfloat16
NPBF = ml_dtypes.bfloat16

NG = 420                 # nodes per graph
G = 2                    # graphs per core
NPC = G * NG             # nodes per core
NCORES = 8
F = 2048                 # input dim
H = 2                    # conv1 heads
D1 = 1024                # conv1 per-head dim
D2 = 64                  # conv2 dim
OUT = 18
FC_K = NG * D2           # 26880
FC_CH = FC_K // 128      # 210
FC_HALF = FC_CH // 2     # 105
SC1 = float(1.0 / np.sqrt(D1))
SC2 = float(1.0 / np.sqrt(D2))

NCH = [(0, 128), (128, 256), (256, 384), (384, 420)]
FCH = F // 128           # 16
DCH = D1 // 128          # 8

Exp = mybir.ActivationFunctionType.Exp
Relu = mybir.ActivationFunctionType.Relu
Copy = mybir.ActivationFunctionType.Copy
Mult = mybir.AluOpType.mult
Add = mybir.AluOpType.add
Max = mybir.AluOpType.max
AxX = mybir.AxisListType.X


def _masked_exp_T(nc, pool, sp, Mti, ssz, scale, si, sfx):
    """S^T chunk psum [ssz,420] -> wt = M^T * exp(scale*S^T) (unnormalized).

    No max-subtraction: scores are O(1) here, exp is safe in fp32. The
    transposed orientation makes the per-dst normalization a column scale,
    done later via a ones-matmul denominator + PE row-replication.
    """
    ex = pool.tile([ssz, NG], F32, tag="ex" + sfx, name="ex")
    nc.scalar.activation(ex[:], sp[:], Exp, scale=scale)
    wt = pool.tile([ssz, NG], F32R, tag=f"wt{si}" + sfx, name="wt")
    nc.vector.tensor_tensor(wt[:], Mti, ex[:], Mult)
    return wt


def _norm_dn(nc, pool, psp, ones, wts, sfx):
    """Stage 1: column-sum the 4 wt chunks via ones-matmuls -> 1/denom."""
    dn = psp.tile([1, NG], F32, tag="dn" + sfx, name="dn")
    for si, (s0, s1) in enumerate(NCH):
        nc.tensor.matmul(dn[:], ones[0:s1 - s0, 0:1], wts[si][:],
                         start=(si == 0), stop=(si == 3))
    dn2 = pool.tile([1, NG], F32, tag="dnb" + sfx, name="dnb")
    nc.vector.tensor_scalar_add(dn2[:], dn[:], 1e-16)
    rcp = pool.tile([1, NG], F32R, tag="rc" + sfx, name="rc")
    with nc.allow_low_precision(reason="f32r is bit-identical to f32 here"):
        nc.vector.reciprocal(rcp[:], dn2[:])
    return rcp


def _norm_rep(nc, psp, ones, rcp, wts, ATt, sfx):
    """Stage 2: replicate 1/denom across partitions, scale the chunks."""
    rep = psp.tile([128, NG], F32, tag="rep" + sfx, name="rep")
    nc.tensor.matmul(rep[:], ones[0:1, :], rcp[:], start=True, stop=True)
    for si, (s0, s1) in enumerate(NCH):
        nc.vector.tensor_tensor(ATt[si][:], wts[si][:], rep[0:s1 - s0, :],
                                Mult)


def _normalize_T(nc, pool, psp, ones, wts, ATt, sfx):
    """Column-normalize the 4 wt chunks: AT[si] = wt[si] / colsum(wt)."""
    rcp = _norm_dn(nc, pool, psp, ones, wts, sfx)
    _norm_rep(nc, psp, ones, rcp, wts, ATt, sfx)


def _build_program():
    nc = bacc.Bacc("TRN2", target_bir_lowering=False, debug=False,
                   num_devices=NCORES)

    def din(name, shape, dt=BF16):
        return nc.dram_tensor(name, shape, dt, kind="ExternalInput")

    hT_d = din("hT", (F, NPC))
    M_d = din("Mm", (128, G * 4 * NG), F32)
    eye_d = din("eye", (128, 128), F32R)
    ones_d = din("ones", (128, 128), F32R)
    wq1_d = din("wq1s", (128, 16 * F))
    wk1_d = din("wk1s", (128, 16 * F))
    ws1_d = din("ws1s", (128, 16 * F))
    wv1_d = din("wv1s", (128, 16 * F))
    w2q_d = din("w2qs", (128, FCH * D2))
    w2k_d = din("w2ks", (128, FCH * D2))
    w2v_d = din("w2vs", (128, FCH * D2))
    w2s_d = din("w2ss", (128, FCH * D2))
    wfc1_d = din("wfc1s", (128, FC_CH * 256))
    wfc2_d = din("wfc2s", (128, 2 * 128))
    wfc3_d = din("wfc3s", (128, 64))
    wfc4_d = din("wfc4s", (64, OUT))
    out_d = nc.dram_tensor("out", (G, OUT), F32, kind="ExternalOutput")

    with tile.TileContext(nc) as tc, ExitStack() as top:
        TP = lambda name, bufs=1, space="SBUF": top.enter_context(
            tc.tile_pool(name=name, bufs=bufs, space=space))
        cstp = TP("cst")
        Mp = TP("Mp")
        r1Tp = TP("r1Tp")
        ATp = TP("ATp")
        w1ap = TP("w1ap")
        r2Tp = TP("r2Tp")

        eye = cstp.tile([128, 128], F32R, name="eye")
        ones = cstp.tile([128, 128], F32R, name="ones")
        Mtile = Mp.tile([128, G * 4 * NG], F32, name="Mtile")

        def Mt(g, ci):
            c0, c1 = NCH[ci]
            blk = (g * 4 + ci) * NG
            return Mtile[0:c1 - c0, blk:blk + NG]

        r1T = [r1Tp.tile([128, NPC], BF16, tag=f"r1T{fc}", name=f"r1T{fc}")
               for fc in range(FCH)]
        AT = {(g, h): [ATp.tile([c1 - c0, NG], BF16, tag=f"AT{g}{h}{c0}",
                                name=f"AT{g}{h}{c0}")
                       for (c0, c1) in NCH]
              for g in range(G) for h in range(H)}
        W1a = w1ap.tile([128, FC_HALF * 256], BF16, name="W1a")

        with tc.tile_pool(name="hTp", bufs=1) as hTp:
            hT = [hTp.tile([128, NPC], BF16, tag=f"hT{fc}", name=f"hT{fc}")
                  for fc in range(FCH)]
            for fc in range(FCH):
                nc.sync.dma_start(hT[fc][:],
                                  hT_d.ap()[fc * 128:(fc + 1) * 128, :])

            # ----- conv1: qT,kT per head -> S -> softmax -> A^T; s1 -----
            with tc.tile_pool(name="slabp", bufs=2) as slabp, \
                 tc.tile_pool(name="qkt", bufs=1) as qkt, \
                 tc.tile_pool(name="qkps", bufs=2, space="PSUM") as qkps, \
                 tc.tile_pool(name="sps", bufs=1, space="PSUM") as sps, \
                 tc.tile_pool(name="smx", bufs=2) as smx:
                qT = [qkt.tile([128, NPC], BF16, tag=f"qT{dc}", name=f"qT{dc}")
                      for dc in range(DCH)]
                kT = [qkt.tile([128, NPC], BF16, tag=f"kT{dc}", name=f"kT{dc}")
                      for dc in range(DCH)]
                # software-pipelined normalization: the dn/rep stages of
                # block (g,h) are emitted inside the NEXT block's si loop so
                # their DVE latency hides under queued PE work
                pend = {}
                for h in range(H):
                    for name_d, dstT in ((wq1_d, qT), (wk1_d, kT)):
                        for dc in range(DCH):
                            slab = slabp.tile([128, F], BF16, tag="slab",
                                              name="slab")
                            dcg = h * DCH + dc
                            nc.sync.dma_start(
                                slab[:],
                                name_d.ap()[:, dcg * F:(dcg + 1) * F])
                            if h == 0 and dc == 2 and name_d is wq1_d:
                                # demoted constant loads, gated on early
                                # projection output so they can't hoist to
                                # t=0 ahead of the slab/hT DMAs
                                nc.vector.tensor_copy(Mtile[0:1, 0:1],
                                                      qT[0][0:1, 0:1])
                                nc.sync.dma_start(Mtile[:], M_d.ap()[:])
                                nc.vector.tensor_copy(ones[0:1, 0:1],
                                                      qT[0][0:1, 0:1])
                                nc.sync.dma_start(ones[:], ones_d.ap()[:])
                            ps = [qkps.tile([128, NG], F32, tag=f"qk{g}",
                                            name=f"qk{g}") for g in range(G)]
                            for fc in range(FCH):
                                for g in range(G):
                                    nc.tensor.matmul(
                                        ps[g][:],
                                        slab[:, fc * 128:(fc + 1) * 128],
                                        hT[fc][:, g * NG:(g + 1) * NG],
                                        start=(fc == 0), stop=(fc == FCH - 1))
                            for g in range(G):
                                nc.scalar.activation(
                                    dstT[dc][:, g * NG:(g + 1) * NG],
                                    ps[g][:], Copy)
                    for g in range(G):
                        wts = []
                        for si, (s0, s1) in enumerate(NCH):
                            ssz = s1 - s0
                            sp = sps.tile([ssz, NG], F32, tag="sp", name="sp")
                            for dc in range(DCH):
                                nc.tensor.matmul(
                                    sp[:],
                                    kT[dc][:, g * NG + s0:g * NG + s1],
                                    qT[dc][:, g * NG:(g + 1) * NG],
                                    start=(dc == 0), stop=(dc == DCH - 1))
                            wts.append(_masked_exp_T(nc, smx, sp, Mt(g, si),
                                                     ssz, SC1, si, "1"))
                            # s1T chunk interleaved here: dense PE work that
                            # fills the softmax DVE/ACT gap
                            dcS = h * DCH + g * 4 + si
                            slab = slabp.tile([128, F], BF16, tag="slab",
                                              name="slab")
                            nc.sync.dma_start(
                                slab[:],
                                ws1_d.ap()[:, dcS * F:(dcS + 1) * F])
                            pss1 = [qkps.tile([128, NG], F32, tag=f"qk{g2}",
                                              name=f"s1{g2}")
                                    for g2 in range(G)]
                            for fc in range(FCH):
                                for g2 in range(G):
                                    nc.tensor.matmul(
                                        pss1[g2][:],
                                        slab[:, fc * 128:(fc + 1) * 128],
                                        hT[fc][:, g2 * NG:(g2 + 1) * NG],
                                        start=(fc == 0), stop=(fc == FCH - 1))
                            for g2 in range(G):
                                nc.scalar.activation(
                                    r1T[dcS][:, g2 * NG:(g2 + 1) * NG],
                                    pss1[g2][:], Copy)
                            if si == 1 and pend:
                                pend["rcp"] = _norm_dn(nc, smx, sps, ones,
                                                       pend["wts"], "1")
                            if si == 2 and pend:
                                _norm_rep(nc, sps, ones, pend["rcp"],
                                          pend["wts"], pend["AT"], "1")
                                pend = {}
                        pend = {"wts": wts, "AT": AT[(g, h)]}
                    if h == 0:
                        # prefetch first half of fc1 weights; the corner
                        # write gates the DMA on mid-conv1 data so it can't
                        # hoist to t=0 and starve the projection slab loads
                        nc.vector.tensor_copy(W1a[0:1, 0:1], qT[0][0:1, 0:1])
                        nc.sync.dma_start(W1a[:],
                                          wfc1_d.ap()[:, :FC_HALF * 256])
                _normalize_T(nc, smx, sps, ones, pend["wts"], pend["AT"],
                             "1")

            # ----- conv1: per-head v then msgT (adds into r1T) -----
            # vt tiles are split by 512-col half so the first msg matmuls
            # (dc<4, half 0) don't wait on the half-1 PSUM copy-outs.
            def do_msg(mgp, vt, h):
                for g in range(G):
                    for dc in range(DCH):
                        mg = mgp.tile([128, NG], F32, tag="mg", name="mg")
                        for si in range(4):
                            nc.tensor.matmul(
                                mg[:],
                                vt[(g, si, dc // 4)][:, (dc % 4) * 128:
                                                     (dc % 4 + 1) * 128],
                                AT[(g, h)][si][:],
                                start=(si == 0), stop=(si == 3))
                        dst = r1T[h * DCH + dc][:, g * NG:(g + 1) * NG]
                        nc.vector.tensor_tensor(dst, dst, mg[:], Add)

            for h in range(H):
                with tc.tile_pool(name="vtp", bufs=1) as vtp:
                    vt = {(g, ci, half): vtp.tile(
                             [c1 - c0, 512], BF16,
                             tag=f"v{g}_{c0}_{half}", name=f"v{g}_{c0}_{half}")
                          for (ci, (c0, c1)) in enumerate(NCH)
                          for g in range(G) for half in range(2)}
                    with tc.tile_pool(name="wvld", bufs=4) as wvld, \
                         tc.tile_pool(name="vps", bufs=1,
                                      space="PSUM") as vps:
                        for half in range(2):
                            pss = {}
                            for g in range(G):
                                for ci, (c0, c1) in enumerate(NCH):
                                    pss[(g, ci)] = vps.tile(
                                        [c1 - c0, 512], F32,
                                        tag=f"vp{g}{ci}", name=f"vp{g}{ci}")
                            for fc in range(FCH):
                                w = wvld.tile([128, 512], BF16, tag="w",
                                              name="w")
                                coff = (h * 2 + half) * (FCH * 512)
                                nc.sync.dma_start(
                                    w[:],
                                    wv1_d.ap()[:, coff + fc * 512:
                                               coff + (fc + 1) * 512])
                                for g in range(G):
                                    for ci, (c0, c1) in enumerate(NCH):
                                        nc.tensor.matmul(
                                            pss[(g, ci)][:],
                                            hT[fc][:, g * NG + c0:
                                                   g * NG + c1],
                                            w[:], start=(fc == 0),
                                            stop=(fc == FCH - 1))
                            for g in range(G):
                                for ci in range(4):
                                    nc.vector.tensor_copy(
                                        vt[(g, ci, half)][:],
                                        pss[(g, ci)][:])
                    with tc.tile_pool(name="mgp", bufs=2,
                                      space="PSUM") as mgp:
                        do_msg(mgp, vt, h)
                # r1T chunks of this head are final: relu them now so
                # conv2 isn't gated on a serial 16-op relu pass later
                for dc in range(DCH):
                    fcr = h * DCH + dc
                    nc.scalar.activation(r1T[fcr][:], r1T[fcr][:], Relu)

        with tc.tile_pool(name="w1bp", bufs=1) as w1bp, \
             tc.tile_pool(name="fcp", bufs=1) as fcp, \
             tc.tile_pool(name="fcw", bufs=1) as fcw:
            W1b = w1bp.tile([128, FC_HALF * 256], BF16, name="W1b")
            nc.vector.tensor_copy(W1b[0:1, 0:1], r1T[8][0:1, 0:1])
            nc.sync.dma_start(W1b[:], wfc1_d.ap()[:, FC_HALF * 256:])
            nc.vector.tensor_copy(eye[0:1, 0:1], r1T[8][0:1, 0:1])
            nc.sync.dma_start(eye[:], eye_d.ap()[:])
            fcin = fcp.tile([128, 2 * FC_CH], BF16, tag="fcin", name="fcin")
            fcin3 = fcin[:].rearrange("p (c t) -> p t c", t=2)

            # ----- conv2 -----
            r2T = r2Tp.tile([D2, NPC], F32R, name="t")
            with tc.tile_pool(name="w2p", bufs=1) as w2p, \
                 tc.tile_pool(name="c2s", bufs=1) as c2s, \
                 tc.tile_pool(name="c2k", bufs=1) as c2k, \
                 tc.tile_pool(name="c2ps", bufs=1, space="PSUM") as c2ps:
                w2t = {}
                for nm, wd in (("q", w2q_d), ("k", w2k_d), ("v", w2v_d),
                               ("s", w2s_d)):
                    tl = w2p.tile([128, FCH * D2], BF16, tag=f"w2{nm}",
                                  name=f"w2{nm}")
                    nc.sync.dma_start(tl[:], wd.ap()[:])
                    w2t[nm] = tl
                qT2 = c2k.tile([D2, NPC], BF16, tag="qT2", name="qT2")
                kT2 = c2k.tile([D2, NPC], BF16, tag="kT2", name="kT2")
                vT2 = c2k.tile([D2, NPC], F32R, tag="vT2", name="vT2")
                for g in range(G):
                    for nm, dstT in (("q", qT2), ("k", kT2), ("v", vT2)):
                        ps = c2ps.tile([D2, NG], F32, tag="p2", name="p2")
                        for fc in range(FCH):
                            nc.tensor.matmul(
                                ps[:], w2t[nm][:, fc * D2:(fc + 1) * D2],
                                r1T[fc][:, g * NG:(g + 1) * NG],
                                start=(fc == 0), stop=(fc == FCH - 1))
                        nc.vector.tensor_copy(dstT[:, g * NG:(g + 1) * NG],
                                              ps[:])
                    ps = c2ps.tile([D2, NG], F32, tag="p2", name="p2")
                    for fc in range(FCH):
                        nc.tensor.matmul(
                            ps[:], w2t["s"][:, fc * D2:(fc + 1) * D2],
                            r1T[fc][:, g * NG:(g + 1) * NG],
                            start=(fc == 0), stop=(fc == FCH - 1))
                    nc.vector.tensor_copy(r2T[:, g * NG:(g + 1) * NG], ps[:])
                v2 = {g: [c2k.tile([c1 - c0, D2], BF16, tag=f"v2{g}_{c0}",
                                   name=f"v2{g}_{c0}")
                          for (c0, c1) in NCH] for g in range(G)}
                for g in range(G):
                    for ci, (c0, c1) in enumerate(NCH):
                        csz = c1 - c0
                        tp_ = c2ps.tile([128, D2], F32R, tag="tp2",
                                        name="tp2")
                        nc.tensor.transpose(tp_[:csz, :],
                                            vT2[:, g * NG + c0:g * NG + c1],
                                            eye[:D2, :D2])
                        nc.vector.tensor_copy(v2[g][ci][:], tp_[:csz, :])
                for g in range(G):
                    a2t = [c2k.tile([c1 - c0, NG], BF16, tag=f"a2t{c0}",
                                    name=f"a2t{c0}")
                           for (c0, c1) in NCH]
                    wts = []
                    for si, (s0, s1) in enumerate(NCH):
                        ssz = s1 - s0
                        sp = c2ps.tile([ssz, NG], F32, tag="sp2", name="sp2")
                        nc.tensor.matmul(sp[:],
                                         kT2[:, g * NG + s0:g * NG + s1],
                                         qT2[:, g * NG:(g + 1) * NG],
                                         start=True, stop=True)
                        wts.append(_masked_exp_T(nc, c2s, sp, Mt(g, si),
                                                 ssz, SC2, si, "2"))
                    _normalize_T(nc, c2s, c2ps, ones, wts, a2t, "2")
                    mg = c2ps.tile([D2, NG], F32, tag="mg2", name="mg2")
                    for si in range(4):
                        nc.tensor.matmul(mg[:], v2[g][si][:], a2t[si][:],
                                         start=(si == 0), stop=(si == 3))
                    dst = r2T[:, g * NG:(g + 1) * NG]
                    nc.vector.tensor_tensor(dst, dst, mg[:], Add)
                    # this graph's r2T is final: relu + gather into fcin now
                    # so fc1 isn't gated on a serial tail
                    nc.scalar.activation(dst, dst, Relu)
                    for par in range(2):
                        src3 = (r2T[:, g * NG:(g + 1) * NG]
                                .rearrange("p (c t) -> p t c", t=2)
                                [:, par:par + 1, :])
                        eng = nc.gpsimd if par == 0 else nc.vector
                        eng.tensor_copy(
                            fcin3[par * 64:(par + 1) * 64, g:g + 1, :], src3)

                # fc1 accumulation lives in the conv2 psum scope so no
                # pool-transition barrier stalls the PE before it.
                # 4-way col-group tiling — four independent K-chunks
                # accumulate concurrently on distinct 32-col strips of the
                # PE array, summed afterwards on DVE
                QS = [(0, 53), (53, 106), (106, 158), (158, 210)]
                f1ps = c2ps.tile([128, 256], F32, tag="f1", name="f1")
                for j in range(53):
                    for qi, (a0, a1) in enumerate(QS):
                        cc = a0 + j
                        if cc >= a1:
                            continue
                        wsrc = W1a if cc < FC_HALF else W1b
                        col = (cc % FC_HALF) * 256
                        nc.tensor.matmul(
                            f1ps[32 * qi:32 * qi + G, :],
                            fcin[:, 2 * cc:2 * cc + 2],
                            wsrc[:, col:col + 256],
                            start=(cc == a0), stop=(cc == a1 - 1),
                            tile_position=(0, 32 * qi),
                            skip_group_check=True)
                # DVE may read only one PSUM operand per op: chain the adds
                s0 = fcp.tile([G, 256], F32, tag="s0", name="s0")
                nc.vector.tensor_copy(s0[:], f1ps[0:G, :])
                s01 = fcp.tile([G, 256], F32, tag="s01", name="s01")
                nc.vector.tensor_tensor(s01[:], s0[:], f1ps[32:32 + G, :],
                                        Add)
                s012 = fcp.tile([G, 256], F32, tag="s012", name="s012")
                nc.vector.tensor_tensor(s012[:], s01[:], f1ps[64:64 + G, :],
                                        Add)
                f1pre = fcp.tile([G, 256], F32, tag="f1p", name="f1p")
                nc.vector.tensor_tensor(f1pre[:], s012[:], f1ps[96:96 + G, :],
                                        Add)
                f1 = fcp.tile([G, 256], F32R, tag="f1s", name="f1s")
                nc.scalar.activation(f1[:], f1pre[:], Relu)

            # ----- tiny fc tail (conv2 pools closed; transition barrier
            # overlaps the f1 DVE/ACT chain) -----
            with tc.tile_pool(name="fps", bufs=1, space="PSUM") as fps:
                f1T = fcp.tile([128, 2 * G], BF16, tag="f1T", name="f1T")
                for half in range(2):
                    tp_ = fps.tile([128, G], F32R, tag="f1tp", name="f1tp")
                    nc.tensor.transpose(
                        tp_[:, :], f1[:, half * 128:(half + 1) * 128],
                        eye[:G, :G])
                    nc.scalar.activation(f1T[:, half * G:(half + 1) * G],
                                         tp_[:], Copy)
                w2 = fcw.tile([128, 2 * 128], BF16, tag="wfc2", name="wfc2")
                nc.sync.dma_start(w2[:], wfc2_d.ap()[:])
                f2ps = fps.tile([128, G], F32, tag="f2", name="f2")
                for half in range(2):
                    nc.tensor.matmul(f2ps[:],
                                     w2[:, half * 128:(half + 1) * 128],
                                     f1T[:, half * G:(half + 1) * G],
                                     start=(half == 0), stop=(half == 1))
                f2T = fcp.tile([128, G], BF16, tag="f2T", name="f2T")
                nc.scalar.activation(f2T[:], f2ps[:], Relu)
                w3 = fcw.tile([128, 64], BF16, tag="wfc3", name="wfc3")
                nc.sync.dma_start(w3[:], wfc3_d.ap()[:])
                f3ps = fps.tile([64, G], F32, tag="f3", name="f3")
                nc.tensor.matmul(f3ps[:], w3[:], f2T[:], start=True,
                                 stop=True)
                f3T = fcp.tile([64, G], BF16, tag="f3T", name="f3T")
                nc.scalar.activation(f3T[:], f3ps[:], Relu)
                w4 = fcw.tile([64, OUT], BF16, tag="wfc4", name="wfc4")
                nc.sync.dma_start(w4[:], wfc4_d.ap()[:])
                f4ps = fps.tile([G, OUT], F32, tag="f4", name="f4")
                nc.tensor.matmul(f4ps[:], f3T[:], w4[:], start=True,
                                 stop=True)
                res = fcp.tile([G, OUT], F32, tag="res", name="res")
                nc.vector.tensor_copy(res[:], f4ps[:])
                nc.sync.dma_start(out_d.ap()[:], res[:])

    nc.compile()
    return nc


_CACHE = {}


def _get_program():
    if "nc" not in _CACHE:
        _CACHE["nc"] = _build_program()
    return _CACHE["nc"]


def _bf(a):
    return np.ascontiguousarray(np.asarray(a, np.float32).astype(NPBF))


def _shard_inputs(inputs):
    x = np.asarray(inputs["x"], dtype=np.float32)
    ei = np.asarray(inputs["edge_index"])
    conn = np.asarray(inputs["connectivity"]).astype(np.int64)
    node_pe = np.asarray(inputs["node_pe"], np.float32)
    lobe = np.asarray(inputs["lobe_pe"], np.float32)
    lung = np.asarray(inputs["lung_pe"], np.float32)

    src, dst = ei[0].astype(np.int64), ei[1].astype(np.int64)
    g_of_e = dst // NG

    def swz(W, pr, blk, inner):
        # W [pr*128, blk*inner] -> [128, blk*pr*inner] with col layout
        # b*(pr*inner) + a*inner + n  == W[a*128+p, b*inner+n]
        W = np.asarray(W, np.float32)
        t = W.reshape(pr, 128, blk, inner).transpose(1, 2, 0, 3)
        return _bf(t.reshape(128, blk * pr * inner))

    shared = {
        "eye": np.eye(128, dtype=np.float32),
        "ones": np.ones((128, 128), dtype=np.float32),
        "wq1s": swz(inputs["Wq1"], 16, 16, 128),
        "wk1s": swz(inputs["Wk1"], 16, 16, 128),
        "ws1s": swz(inputs["Ws1"], 16, 16, 128),
        "wv1s": swz(inputs["Wv1"], 16, 4, 512),
        "w2qs": swz(inputs["Wq2"], 16, 1, 64),
        "w2ks": swz(inputs["Wk2"], 16, 1, 64),
        "w2vs": swz(inputs["Wv2"], 16, 1, 64),
        "w2ss": swz(inputs["Ws2"], 16, 1, 64),
        "wfc1s": swz(inputs["W_fc1"], FC_CH, 1, 256),
        "wfc2s": swz(inputs["W_fc2"], 2, 1, 128),
        "wfc3s": _bf(inputs["W_fc3"]),
        "wfc4s": _bf(inputs["W_fc4"]),
    }

    in_maps = []
    for c in range(NCORES):
        m = dict(shared)
        sl = slice(c * NPC, (c + 1) * NPC)
        cc = conn[sl]
        h = (x[sl] + np.tile(node_pe, (G, 1))
             + lobe[cc - 1] + lung[(cc > 2).astype(np.int64)])
        m["hT"] = _bf(h.T)
        Mp = np.zeros((128, G * 4 * NG), np.float32)
        for s in range(G):
            gid = G * c + s
            idx = np.nonzero(g_of_e == gid)[0]
            # transposed: rows = src chunk, cols = dst (S^T orientation)
            Mg = np.zeros((NG, NG), np.float32)
            np.add.at(Mg, (src[idx] - NG * gid, dst[idx] - NG * gid), 1.0)
            for si, (s0, s1) in enumerate(NCH):
                blk = (s * 4 + si) * NG
                Mp[0:s1 - s0, blk:blk + NG] = Mg[s0:s1, :]
        m["Mm"] = np.ascontiguousarray(Mp)
        in_maps.append(m)
    return in_maps


def kernel(**inputs):
    nc = _get_program()
    in_maps = _shard_inputs(inputs)
    res = bass_utils.run_bass_kernel_spmd(
        nc, in_maps, core_ids=list(range(NCORES)))
    out = np.concatenate([r["out"] for r in res.results], axis=0)
    return out.astype(np.float32)


def run_traced(inputs, trace_cores=None, stitch=False):
    """Testing entry: returns (output, BassKernelResults incl. trace)."""
    nc = _get_program()
    in_maps = _shard_inputs(inputs)
    res = bass_utils.run_bass_kernel_spmd(
        nc, in_maps, core_ids=list(range(NCORES)), trace=True,
        trace_cores=trace_cores, stitch_traces=stitch)
    out = np.concatenate([r["out"] for r in res.results], axis=0)
    return out.astype(np.float32), res


# revision 38
# speedup vs baseline: 1.0915x; 1.0915x over previous
"""Trainium2 Bass kernel for nn_GraphTransformerPE.

Sharding: graph-data-parallel. 16 graphs x 420 nodes; core c owns graphs
(2c, 2c+1). Weights replicated, no cross-core traffic; host slices inputs,
precomputes hT = (x + node/lobe/lung PE)^T and the per-graph edge-count
matrices M, pre-swizzles all weights into their SBUF slab layouts (all in
bf16), and concatenates the per-core [2,18] outputs.

Device formulation: per-graph DENSE attention. M is the 420x420 edge
multiplicity matrix, then TransformerConv softmax-aggregation ==
  w = M * exp(S/sqrt(d) - rowmax),  A = w / (rowsum(w)+1e-16),
  msg = A @ V  (computed transposed),
which reproduces segment softmax exactly. Matmuls use bf16 stationary
operands (weights / hT / vt) to enable fast-weight-load; accumulation is
always fp32 in PSUM. Biases are all zero in this model and skipped.

Layout: activations feature-major (transposed): hT [2048,840] bf16 feeds
every projection; conv outputs produced directly transposed (r1T
[2048,840] bf16, r2T [64,840]); fc1 weights are prefetched into SBUF
(bf16) during conv1/conv2 so the fc head runs without DMA waits.
"""

import sys
import types
from contextlib import ExitStack

import numpy as np
import ml_dtypes

# ---- NTFF profile hook shim (antenv.axon_hooks absent in this image) ----
if "antenv.axon_hooks" not in sys.modules:
    _m = types.ModuleType("antenv.axon_hooks")
    _hook = [None]
    _m.set_axon_ntff_profile_hook = lambda h: _hook.__setitem__(0, h)
    _m.get_axon_ntff_profile_hook = lambda: _hook[0]
    sys.modules["antenv.axon_hooks"] = _m
    try:
        from trn_agent_boot.trn_boot import _ntff_profile_via_ctypes
        _m.set_axon_ntff_profile_hook(
            _ntff_profile_via_ctypes("/opt/axon/libaxon_pjrt.so"))
    except Exception:
        pass

import concourse.bacc as bacc
import concourse.tile as tile
from concourse import bass_utils, mybir

F32 = mybir.dt.float32
F32R = mybir.dt.float32r
BF16 = mybir.dt.bfloat16
NPBF = ml_dtypes.bfloat16

NG = 420                 # nodes per graph
G = 2                    # graphs per core
NPC = G * NG             # nodes per core
NCORES = 8
F = 2048                 # input dim
H = 2                    # conv1 heads
D1 = 1024                # conv1 per-head dim
D2 = 64                  # conv2 dim
OUT = 18
FC_K = NG * D2           # 26880
FC_CH = FC_K // 128      # 210
FC_HALF = FC_CH // 2     # 105
SC1 = float(1.0 / np.sqrt(D1))
SC2 = float(1.0 / np.sqrt(D2))

NCH = [(0, 128), (128, 256), (256, 384), (384, 420)]
FCH = F // 128           # 16
DCH = D1 // 128          # 8

Exp = mybir.ActivationFunctionType.Exp
Relu = mybir.ActivationFunctionType.Relu
Copy = mybir.ActivationFunctionType.Copy
Mult = mybir.AluOpType.mult
Add = mybir.AluOpType.add
Max = mybir.AluOpType.max
AxX = mybir.AxisListType.X


def _masked_exp_T(nc, pool, sp, Mti, ssz, scale, si, sfx):
    """S^T chunk psum [ssz,420] -> wt = M^T * exp(scale*S^T) (unnormalized).

    No max-subtraction: scores are O(1) here, exp is safe in fp32. The
    transposed orientation makes the per-dst normalization a column scale,
    done later via a ones-matmul denominator + PE row-replication.
    """
    ex = pool.tile([ssz, NG], F32, tag="ex" + sfx, name="ex")
    nc.scalar.activation(ex[:], sp[:], Exp, scale=scale)
    wt = pool.tile([ssz, NG], F32R, tag=f"wt{si}" + sfx, name="wt")
    nc.vector.tensor_tensor(wt[:], Mti, ex[:], Mult)
    return wt


def _norm_dn(nc, pool, psp, ones, wts, sfx):
    """Stage 1: column-sum the 4 wt chunks via ones-matmuls -> 1/denom."""
    dn = psp.tile([1, NG], F32, tag="dn" + sfx, name="dn")
    for si, (s0, s1) in enumerate(NCH):
        nc.tensor.matmul(dn[:], ones[0:s1 - s0, 0:1], wts[si][:],
                         start=(si == 0), stop=(si == 3))
    dn2 = pool.tile([1, NG], F32, tag="dnb" + sfx, name="dnb")
    nc.vector.tensor_scalar_add(dn2[:], dn[:], 1e-16)
    rcp = pool.tile([1, NG], F32R, tag="rc" + sfx, name="rc")
    with nc.allow_low_precision(reason="f32r is bit-identical to f32 here"):
        nc.vector.reciprocal(rcp[:], dn2[:])
    return rcp


def _norm_rep(nc, psp, ones, rcp, wts, ATt, sfx):
    """Stage 2: replicate 1/denom across partitions, scale the chunks."""
    rep = psp.tile([128, NG], F32, tag="rep" + sfx, name="rep")
    nc.tensor.matmul(rep[:], ones[0:1, :], rcp[:], start=True, stop=True)
    for si, (s0, s1) in enumerate(NCH):
        nc.vector.tensor_tensor(ATt[si][:], wts[si][:], rep[0:s1 - s0, :],
                                Mult)


def _normalize_T(nc, pool, psp, ones, wts, ATt, sfx):
    """Column-normalize the 4 wt chunks: AT[si] = wt[si] / colsum(wt)."""
    rcp = _norm_dn(nc, pool, psp, ones, wts, sfx)
    _norm_rep(nc, psp, ones, rcp, wts, ATt, sfx)


def _build_program():
    nc = bacc.Bacc("TRN2", target_bir_lowering=False, debug=False,
                   num_devices=NCORES)

    def din(name, shape, dt=BF16):
        return nc.dram_tensor(name, shape, dt, kind="ExternalInput")

    hT_d = din("hT", (F, NPC))
    M_d = din("Mm", (128, G * 4 * NG), F32)
    eye_d = din("eye", (128, 128), F32R)
    ones_d = din("ones", (128, 128), F32R)
    wq1_d = din("wq1s", (128, 16 * F))
    wk1_d = din("wk1s", (128, 16 * F))
    ws1_d = din("ws1s", (128, 16 * F))
    wv1_d = din("wv1s", (128, 16 * F))
    w2q_d = din("w2qs", (128, FCH * D2))
    w2k_d = din("w2ks", (128, FCH * D2))
    w2v_d = din("w2vs", (128, FCH * D2))
    w2s_d = din("w2ss", (128, FCH * D2))
    wfc1_d = din("wfc1s", (128, FC_CH * 256))
    wfc2_d = din("wfc2s", (128, 2 * 128))
    wfc3_d = din("wfc3s", (128, 64))
    wfc4_d = din("wfc4s", (64, OUT))
    out_d = nc.dram_tensor("out", (G, OUT), F32, kind="ExternalOutput")

    with tile.TileContext(nc) as tc, ExitStack() as top:
        TP = lambda name, bufs=1, space="SBUF": top.enter_context(
            tc.tile_pool(name=name, bufs=bufs, space=space))
        cstp = TP("cst")
        Mp = TP("Mp")
        r1Tp = TP("r1Tp")
        ATp = TP("ATp")
        w1ap = TP("w1ap")
        r2Tp = TP("r2Tp")

        eye = cstp.tile([128, 128], F32R, name="eye")
        ones = cstp.tile([128, 128], F32R, name="ones")
        Mtile = Mp.tile([128, G * 4 * NG], F32, name="Mtile")

        def Mt(g, ci):
            c0, c1 = NCH[ci]
            blk = (g * 4 + ci) * NG
            return Mtile[0:c1 - c0, blk:blk + NG]

        r1T = [r1Tp.tile([128, NPC], BF16, tag=f"r1T{fc}", name=f"r1T{fc}")
               for fc in range(FCH)]
        AT = {(g, h): [ATp.tile([c1 - c0, NG], BF16, tag=f"AT{g}{h}{c0}",
                                name=f"AT{g}{h}{c0}")
                       for (c0, c1) in NCH]
              for g in range(G) for h in range(H)}
        W1a = w1ap.tile([128, FC_HALF * 256], BF16, name="W1a")

        with tc.tile_pool(name="hTp", bufs=1) as hTp:
            hT = [hTp.tile([128, NPC], BF16, tag=f"hT{fc}", name=f"hT{fc}")
                  for fc in range(FCH)]
            # only the first 4 hT chunks load un-gated: the rest are gated
            # behind the first weight slab so fair-share DMA scheduling
            # doesn't delay the first matmul by the whole 3.4MB transfer
            for fc in range(4):
                nc.sync.dma_start(hT[fc][:],
                                  hT_d.ap()[fc * 128:(fc + 1) * 128, :])

            # ----- conv1: qT,kT per head -> S -> softmax -> A^T; s1 -----
            with tc.tile_pool(name="slabp", bufs=2) as slabp, \
                 tc.tile_pool(name="qkt", bufs=1) as qkt, \
                 tc.tile_pool(name="qkps", bufs=2, space="PSUM") as qkps, \
                 tc.tile_pool(name="sps", bufs=1, space="PSUM") as sps, \
                 tc.tile_pool(name="smx", bufs=2) as smx:
                qT = [qkt.tile([128, NPC], BF16, tag=f"qT{dc}", name=f"qT{dc}")
                      for dc in range(DCH)]
                kT = [qkt.tile([128, NPC], BF16, tag=f"kT{dc}", name=f"kT{dc}")
                      for dc in range(DCH)]
                # software-pipelined normalization: the dn/rep stages of
                # block (g,h) are emitted inside the NEXT block's si loop so
                # their DVE latency hides under queued PE work
                pend = {}
                for h in range(H):
                    for name_d, dstT in ((wq1_d, qT), (wk1_d, kT)):
                        for dc in range(DCH):
                            slab = slabp.tile([128, F], BF16, tag="slab",
                                              name="slab")
                            dcg = h * DCH + dc
                            nc.sync.dma_start(
                                slab[:],
                                name_d.ap()[:, dcg * F:(dcg + 1) * F])
                            if h == 0 and dc == 0 and name_d is wq1_d:
                                for fc in range(4, FCH):
                                    nc.vector.tensor_copy(hT[fc][0:1, 0:1],
                                                          slab[0:1, 0:1])
                                    nc.sync.dma_start(
                                        hT[fc][:],
                                        hT_d.ap()[fc * 128:(fc + 1) * 128, :])
                            if h == 0 and dc == 2 and name_d is wq1_d:
                                # demoted constant loads, gated on early
                                # projection output so they can't hoist to
                                # t=0 ahead of the slab/hT DMAs
                                nc.vector.tensor_copy(Mtile[0:1, 0:1],
                                                      qT[0][0:1, 0:1])
                                nc.sync.dma_start(Mtile[:], M_d.ap()[:])
                                nc.vector.tensor_copy(ones[0:1, 0:1],
                                                      qT[0][0:1, 0:1])
                                nc.sync.dma_start(ones[:], ones_d.ap()[:])
                            ps = [qkps.tile([128, NG], F32, tag=f"qk{g}",
                                            name=f"qk{g}") for g in range(G)]
                            for fc in range(FCH):
                                for g in range(G):
                                    nc.tensor.matmul(
                                        ps[g][:],
                                        slab[:, fc * 128:(fc + 1) * 128],
                                        hT[fc][:, g * NG:(g + 1) * NG],
                                        start=(fc == 0), stop=(fc == FCH - 1))
                            for g in range(G):
                                nc.scalar.activation(
                                    dstT[dc][:, g * NG:(g + 1) * NG],
                                    ps[g][:], Copy)
                    for g in range(G):
                        wts = []
                        for si, (s0, s1) in enumerate(NCH):
                            ssz = s1 - s0
                            sp = sps.tile([ssz, NG], F32, tag="sp", name="sp")
                            for dc in range(DCH):
                                nc.tensor.matmul(
                                    sp[:],
                                    kT[dc][:, g * NG + s0:g * NG + s1],
                                    qT[dc][:, g * NG:(g + 1) * NG],
                                    start=(dc == 0), stop=(dc == DCH - 1))
                            wts.append(_masked_exp_T(nc, smx, sp, Mt(g, si),
                                                     ssz, SC1, si, "1"))
                            # s1T chunk interleaved here: dense PE work that
                            # fills the softmax DVE/ACT gap
                            dcS = h * DCH + g * 4 + si
                            slab = slabp.tile([128, F], BF16, tag="slab",
                                              name="slab")
                            nc.sync.dma_start(
                                slab[:],
                                ws1_d.ap()[:, dcS * F:(dcS + 1) * F])
                            pss1 = [qkps.tile([128, NG], F32, tag=f"qk{g2}",
                                              name=f"s1{g2}")
                                    for g2 in range(G)]
                            for fc in range(FCH):
                                for g2 in range(G):
                                    nc.tensor.matmul(
                                        pss1[g2][:],
                                        slab[:, fc * 128:(fc + 1) * 128],
                                        hT[fc][:, g2 * NG:(g2 + 1) * NG],
                                        start=(fc == 0), stop=(fc == FCH - 1))
                            for g2 in range(G):
                                nc.scalar.activation(
                                    r1T[dcS][:, g2 * NG:(g2 + 1) * NG],
                                    pss1[g2][:], Copy)
                            if si == 1 and pend:
                                pend["rcp"] = _norm_dn(nc, smx, sps, ones,
                                                       pend["wts"], "1")
                            if si == 2 and pend:
                                _norm_rep(nc, sps, ones, pend["rcp"],
                                          pend["wts"], pend["AT"], "1")
                                pend = {}
                        pend = {"wts": wts, "AT": AT[(g, h)]}
                    if h == 0:
                        # prefetch first half of fc1 weights; the corner
                        # write gates the DMA on mid-conv1 data so it can't
                        # hoist to t=0 and starve the projection slab loads
                        nc.vector.tensor_copy(W1a[0:1, 0:1], qT[0][0:1, 0:1])
                        nc.sync.dma_start(W1a[:],
                                          wfc1_d.ap()[:, :FC_HALF * 256])
                _normalize_T(nc, smx, sps, ones, pend["wts"], pend["AT"],
                             "1")

            # ----- conv1: per-head v then msgT (adds into r1T) -----
            # vt tiles are split by 512-col half so the first msg matmuls
            # (dc<4, half 0) don't wait on the half-1 PSUM copy-outs.
            def do_msg(mgp, vt, h):
                for g in range(G):
                    for dc in range(DCH):
                        mg = mgp.tile([128, NG], F32, tag="mg", name="mg")
                        for si in range(4):
                            nc.tensor.matmul(
                                mg[:],
                                vt[(g, si, dc // 4)][:, (dc % 4) * 128:
                                                     (dc % 4 + 1) * 128],
                                AT[(g, h)][si][:],
                                start=(si == 0), stop=(si == 3))
                        dst = r1T[h * DCH + dc][:, g * NG:(g + 1) * NG]
                        nc.vector.tensor_tensor(dst, dst, mg[:], Add)

            for h in range(H):
                # v accumulation is per-graph (4 psum banks) so the msg
                # pool co-resides (6 banks total) — no psum pool transition
                # between v and msg, so the PE never idles long enough to
                # re-throttle the HAM clock
                with tc.tile_pool(name="vtp", bufs=1) as vtp, \
                     tc.tile_pool(name="wvp", bufs=1) as wvp, \
                     tc.tile_pool(name="vps", bufs=1, space="PSUM") as vps, \
                     tc.tile_pool(name="mgp", bufs=2, space="PSUM") as mgp:
                    vt = {(g, ci, half): vtp.tile(
                             [c1 - c0, 512], BF16,
                             tag=f"v{g}_{c0}_{half}", name=f"v{g}_{c0}_{half}")
                          for (ci, (c0, c1)) in enumerate(NCH)
                          for g in range(G) for half in range(2)}
                    for half in range(2):
                        wv = []
                        for fc in range(FCH):
                            w = wvp.tile([128, 512], BF16, tag=f"wv{fc}",
                                         name=f"wv{fc}")
                            coff = (h * 2 + half) * (FCH * 512)
                            nc.sync.dma_start(
                                w[:], wv1_d.ap()[:, coff + fc * 512:
                                                 coff + (fc + 1) * 512])
                            wv.append(w)
                        for g in range(G):
                            pss = {ci: vps.tile([c1 - c0, 512], F32,
                                                tag=f"vp{ci}",
                                                name=f"vp{ci}")
                                   for ci, (c0, c1) in enumerate(NCH)}
                            for fc in range(FCH):
                                for ci, (c0, c1) in enumerate(NCH):
                                    nc.tensor.matmul(
                                        pss[ci][:],
                                        hT[fc][:, g * NG + c0:g * NG + c1],
                                        wv[fc][:], start=(fc == 0),
                                        stop=(fc == FCH - 1))
                            for ci in range(4):
                                nc.vector.tensor_copy(
                                    vt[(g, ci, half)][:], pss[ci][:])
                    do_msg(mgp, vt, h)
                # r1T chunks of this head are final: relu them now so
                # conv2 isn't gated on a serial 16-op relu pass later
                for dc in range(DCH):
                    fcr = h * DCH + dc
                    nc.scalar.activation(r1T[fcr][:], r1T[fcr][:], Relu)

        with tc.tile_pool(name="w1bp", bufs=1) as w1bp, \
             tc.tile_pool(name="fcp", bufs=1) as fcp, \
             tc.tile_pool(name="fcw", bufs=1) as fcw:
            W1b = w1bp.tile([128, FC_HALF * 256], BF16, name="W1b")
            nc.vector.tensor_copy(W1b[0:1, 0:1], r1T[8][0:1, 0:1])
            nc.sync.dma_start(W1b[:], wfc1_d.ap()[:, FC_HALF * 256:])
            nc.vector.tensor_copy(eye[0:1, 0:1], r1T[8][0:1, 0:1])
            nc.sync.dma_start(eye[:], eye_d.ap()[:])
            fcin = fcp.tile([128, 2 * FC_CH], BF16, tag="fcin", name="fcin")
            fcin3 = fcin[:].rearrange("p (c t) -> p t c", t=2)

            # ----- conv2 -----
            r2T = r2Tp.tile([D2, NPC], F32R, name="t")
            with tc.tile_pool(name="w2p", bufs=1) as w2p, \
                 tc.tile_pool(name="c2s", bufs=1) as c2s, \
                 tc.tile_pool(name="c2k", bufs=1) as c2k, \
                 tc.tile_pool(name="c2ps", bufs=1, space="PSUM") as c2ps:
                w2t = {}
                for nm, wd in (("q", w2q_d), ("k", w2k_d), ("v", w2v_d),
                               ("s", w2s_d)):
                    tl = w2p.tile([128, FCH * D2], BF16, tag=f"w2{nm}",
                                  name=f"w2{nm}")
                    nc.sync.dma_start(tl[:], wd.ap()[:])
                    w2t[nm] = tl
                qT2 = c2k.tile([D2, NPC], BF16, tag="qT2", name="qT2")
                kT2 = c2k.tile([D2, NPC], BF16, tag="kT2", name="kT2")
                vT2 = c2k.tile([D2, NPC], F32R, tag="vT2", name="vT2")
                for g in range(G):
                    for nm, dstT in (("q", qT2), ("k", kT2), ("v", vT2)):
                        ps = c2ps.tile([D2, NG], F32, tag="p2", name="p2")
                        for fc in range(FCH):
                            nc.tensor.matmul(
                                ps[:], w2t[nm][:, fc * D2:(fc + 1) * D2],
                                r1T[fc][:, g * NG:(g + 1) * NG],
                                start=(fc == 0), stop=(fc == FCH - 1))
                        nc.vector.tensor_copy(dstT[:, g * NG:(g + 1) * NG],
                                              ps[:])
                    ps = c2ps.tile([D2, NG], F32, tag="p2", name="p2")
                    for fc in range(FCH):
                        nc.tensor.matmul(
                            ps[:], w2t["s"][:, fc * D2:(fc + 1) * D2],
                            r1T[fc][:, g * NG:(g + 1) * NG],
                            start=(fc == 0), stop=(fc == FCH - 1))
                    nc.vector.tensor_copy(r2T[:, g * NG:(g + 1) * NG], ps[:])
                v2 = {g: [c2k.tile([c1 - c0, D2], BF16, tag=f"v2{g}_{c0}",
                                   name=f"v2{g}_{c0}")
                          for (c0, c1) in NCH] for g in range(G)}
                for g in range(G):
                    for ci, (c0, c1) in enumerate(NCH):
                        csz = c1 - c0
                        tp_ = c2ps.tile([128, D2], F32R, tag="tp2",
                                        name="tp2")
                        nc.tensor.transpose(tp_[:csz, :],
                                            vT2[:, g * NG + c0:g * NG + c1],
                                            eye[:D2, :D2])
                        nc.vector.tensor_copy(v2[g][ci][:], tp_[:csz, :])
                for g in range(G):
                    a2t = [c2k.tile([c1 - c0, NG], BF16, tag=f"a2t{c0}",
                                    name=f"a2t{c0}")
                           for (c0, c1) in NCH]
                    wts = []
                    for si, (s0, s1) in enumerate(NCH):
                        ssz = s1 - s0
                        sp = c2ps.tile([ssz, NG], F32, tag="sp2", name="sp2")
                        nc.tensor.matmul(sp[:],
                                         kT2[:, g * NG + s0:g * NG + s1],
                                         qT2[:, g * NG:(g + 1) * NG],
                                         start=True, stop=True)
                        wts.append(_masked_exp_T(nc, c2s, sp, Mt(g, si),
                                                 ssz, SC2, si, "2"))
                    _normalize_T(nc, c2s, c2ps, ones, wts, a2t, "2")
                    mg = c2ps.tile([D2, NG], F32, tag="mg2", name="mg2")
                    for si in range(4):
                        nc.tensor.matmul(mg[:], v2[g][si][:], a2t[si][:],
                                         start=(si == 0), stop=(si == 3))
                    dst = r2T[:, g * NG:(g + 1) * NG]
                    nc.vector.tensor_tensor(dst, dst, mg[:], Add)
                    # this graph's r2T is final: relu + gather into fcin now
                    # so fc1 isn't gated on a serial tail
                    nc.scalar.activation(dst, dst, Relu)
                    for par in range(2):
                        src3 = (r2T[:, g * NG:(g + 1) * NG]
                                .rearrange("p (c t) -> p t c", t=2)
                                [:, par:par + 1, :])
                        eng = nc.gpsimd if par == 0 else nc.vector
                        eng.tensor_copy(
                            fcin3[par * 64:(par + 1) * 64, g:g + 1, :], src3)

                # fc1 accumulation lives in the conv2 psum scope so no
                # pool-transition barrier stalls the PE before it.
                # 4-way col-group tiling — four independent K-chunks
                # accumulate concurrently on distinct 32-col strips of the
                # PE array, summed afterwards on DVE
                QS = [(0, 53), (53, 106), (106, 158), (158, 210)]
                f1ps = c2ps.tile([128, 256], F32, tag="f1", name="f1")
                for j in range(53):
                    for qi, (a0, a1) in enumerate(QS):
                        cc = a0 + j
                        if cc >= a1:
                            continue
                        wsrc = W1a if cc < FC_HALF else W1b
                        col = (cc % FC_HALF) * 256
                        nc.tensor.matmul(
                            f1ps[32 * qi:32 * qi + G, :],
                            fcin[:, 2 * cc:2 * cc + 2],
                            wsrc[:, col:col + 256],
                            start=(cc == a0), stop=(cc == a1 - 1),
                            tile_position=(0, 32 * qi),
                            skip_group_check=True)
                # DVE may read only one PSUM operand per op: chain the adds
                s0 = fcp.tile([G, 256], F32, tag="s0", name="s0")
                nc.vector.tensor_copy(s0[:], f1ps[0:G, :])
                s01 = fcp.tile([G, 256], F32, tag="s01", name="s01")
                nc.vector.tensor_tensor(s01[:], s0[:], f1ps[32:32 + G, :],
                                        Add)
                s012 = fcp.tile([G, 256], F32, tag="s012", name="s012")
                nc.vector.tensor_tensor(s012[:], s01[:], f1ps[64:64 + G, :],
                                        Add)
                f1pre = fcp.tile([G, 256], F32, tag="f1p", name="f1p")
                nc.vector.tensor_tensor(f1pre[:], s012[:], f1ps[96:96 + G, :],
                                        Add)
                f1 = fcp.tile([G, 256], F32R, tag="f1s", name="f1s")
                nc.scalar.activation(f1[:], f1pre[:], Relu)

            # ----- tiny fc tail (conv2 pools closed; transition barrier
            # overlaps the f1 DVE/ACT chain) -----
            with tc.tile_pool(name="fps", bufs=1, space="PSUM") as fps:
                f1T = fcp.tile([128, 2 * G], BF16, tag="f1T", name="f1T")
                for half in range(2):
                    tp_ = fps.tile([128, G], F32R, tag="f1tp", name="f1tp")
                    nc.tensor.transpose(
                        tp_[:, :], f1[:, half * 128:(half + 1) * 128],
                        eye[:G, :G])
                    nc.scalar.activation(f1T[:, half * G:(half + 1) * G],
                                         tp_[:], Copy)
                w2 = fcw.tile([128, 2 * 128], BF16, tag="wfc2", name="wfc2")
                nc.sync.dma_start(w2[:], wfc2_d.ap()[:])
                f2ps = fps.tile([128, G], F32, tag="f2", name="f2")
                for half in range(2):
                    nc.tensor.matmul(f2ps[:],
                                     w2[:, half * 128:(half + 1) * 128],
                                     f1T[:, half * G:(half + 1) * G],
                                     start=(half == 0), stop=(half == 1))
                f2T = fcp.tile([128, G], BF16, tag="f2T", name="f2T")
                nc.scalar.activation(f2T[:], f2ps[:], Relu)
                w3 = fcw.tile([128, 64], BF16, tag="wfc3", name="wfc3")
                nc.sync.dma_start(w3[:], wfc3_d.ap()[:])
                f3ps = fps.tile([64, G], F32, tag="f3", name="f3")
                nc.tensor.matmul(f3ps[:], w3[:], f2T[:], start=True,
                                 stop=True)
                f3T = fcp.tile([64, G], BF16, tag="f3T", name="f3T")
                nc.scalar.activation(f3T[:], f3ps[:], Relu)
                w4 = fcw.tile([64, OUT], BF16, tag="wfc4", name="wfc4")
                nc.sync.dma_start(w4[:], wfc4_d.ap()[:])
                f4ps = fps.tile([G, OUT], F32, tag="f4", name="f4")
                nc.tensor.matmul(f4ps[:], f3T[:], w4[:], start=True,
                                 stop=True)
                res = fcp.tile([G, OUT], F32, tag="res", name="res")
                nc.vector.tensor_copy(res[:], f4ps[:])
                nc.sync.dma_start(out_d.ap()[:], res[:])

    nc.compile()
    return nc


_CACHE = {}


def _get_program():
    if "nc" not in _CACHE:
        _CACHE["nc"] = _build_program()
    return _CACHE["nc"]


def _bf(a):
    return np.ascontiguousarray(np.asarray(a, np.float32).astype(NPBF))


def _shard_inputs(inputs):
    x = np.asarray(inputs["x"], dtype=np.float32)
    ei = np.asarray(inputs["edge_index"])
    conn = np.asarray(inputs["connectivity"]).astype(np.int64)
    node_pe = np.asarray(inputs["node_pe"], np.float32)
    lobe = np.asarray(inputs["lobe_pe"], np.float32)
    lung = np.asarray(inputs["lung_pe"], np.float32)

    src, dst = ei[0].astype(np.int64), ei[1].astype(np.int64)
    g_of_e = dst // NG

    def swz(W, pr, blk, inner):
        # W [pr*128, blk*inner] -> [128, blk*pr*inner] with col layout
        # b*(pr*inner) + a*inner + n  == W[a*128+p, b*inner+n]
        W = np.asarray(W, np.float32)
        t = W.reshape(pr, 128, blk, inner).transpose(1, 2, 0, 3)
        return _bf(t.reshape(128, blk * pr * inner))

    shared = {
        "eye": np.eye(128, dtype=np.float32),
        "ones": np.ones((128, 128), dtype=np.float32),
        "wq1s": swz(inputs["Wq1"], 16, 16, 128),
        "wk1s": swz(inputs["Wk1"], 16, 16, 128),
        "ws1s": swz(inputs["Ws1"], 16, 16, 128),
        "wv1s": swz(inputs["Wv1"], 16, 4, 512),
        "w2qs": swz(inputs["Wq2"], 16, 1, 64),
        "w2ks": swz(inputs["Wk2"], 16, 1, 64),
        "w2vs": swz(inputs["Wv2"], 16, 1, 64),
        "w2ss": swz(inputs["Ws2"], 16, 1, 64),
        "wfc1s": swz(inputs["W_fc1"], FC_CH, 1, 256),
        "wfc2s": swz(inputs["W_fc2"], 2, 1, 128),
        "wfc3s": _bf(inputs["W_fc3"]),
        "wfc4s": _bf(inputs["W_fc4"]),
    }

    in_maps = []
    for c in range(NCORES):
        m = dict(shared)
        sl = slice(c * NPC, (c + 1) * NPC)
        cc = conn[sl]
        h = (x[sl] + np.tile(node_pe, (G, 1))
             + lobe[cc - 1] + lung[(cc > 2).astype(np.int64)])
        m["hT"] = _bf(h.T)
        Mp = np.zeros((128, G * 4 * NG), np.float32)
        for s in range(G):
            gid = G * c + s
            idx = np.nonzero(g_of_e == gid)[0]
            # transposed: rows = src chunk, cols = dst (S^T orientation)
            Mg = np.zeros((NG, NG), np.float32)
            np.add.at(Mg, (src[idx] - NG * gid, dst[idx] - NG * gid), 1.0)
            for si, (s0, s1) in enumerate(NCH):
                blk = (s * 4 + si) * NG
                Mp[0:s1 - s0, blk:blk + NG] = Mg[s0:s1, :]
        m["Mm"] = np.ascontiguousarray(Mp)
        in_maps.append(m)
    return in_maps


def kernel(**inputs):
    nc = _get_program()
    in_maps = _shard_inputs(inputs)
    res = bass_utils.run_bass_kernel_spmd(
        nc, in_maps, core_ids=list(range(NCORES)))
    out = np.concatenate([r["out"] for r in res.results], axis=0)
    return out.astype(np.float32)


def run_traced(inputs, trace_cores=None, stitch=False):
    """Testing entry: returns (output, BassKernelResults incl. trace)."""
    nc = _get_program()
    in_maps = _shard_inputs(inputs)
    res = bass_utils.run_bass_kernel_spmd(
        nc, in_maps, core_ids=list(range(NCORES)), trace=True,
        trace_cores=trace_cores, stitch_traces=stitch)
    out = np.concatenate([r["out"] for r in res.results], axis=0)
    return out.astype(np.float32), res
